# revision 1
# baseline (speedup 1.0000x reference)
# Trainium2 Bass kernel for nn_CustomConv2D_57200374448719:
#   data [32,128,64,64] f32 (NCHW) conv weights [256,128,3,3] (OIHW),
#   VALID, stride 1 -> out [32,256,62,62] f32.
#
# Strategy: data-parallel over batch across 8 NeuronCores (4 images per
# core), weights replicated. Per core, implicit GEMM with C_in=128 on the
# SBUF partition axis: for each image / C_out half (128) / group of 8
# output rows, accumulate 9 matmuls (one per 3x3 tap, K=128, N=rows*62)
# into one PSUM bank. The shifted conv windows are strided access
# patterns on the resident [128, 64*64] image tile, so no im2col copy is
# ever materialized. Matmuls run in float32r (bit-identical fp32 in
# memory, reduced-precision multiply at full PE rate); accumulation is
# fp32 in PSUM.
import numpy as np

N_CORES = 8
B, CIN, H, W = 32, 128, 64, 64
COUT, KH, KW = 256, 3, 3
OH, OW = H - KH + 1, W - KW + 1  # 62, 62
BPC = B // N_CORES  # images per core
ROW_GROUPS = [(r0, min(8, OH - r0)) for r0 in range(0, OH, 8)]  # 7x8 + 1x6

_cache = {}


def build_nc(mm_dtype_name="float32r"):
    import concourse.bacc as bacc
    import concourse.mybir as mybir
    import concourse.tile as tile

    mm_dt = getattr(mybir.dt, mm_dtype_name)
    f32 = mybir.dt.float32

    nc = bacc.Bacc("TRN2", target_bir_lowering=False, debug=False, num_devices=N_CORES)
    data_in = nc.dram_tensor("data", [BPC, CIN, H, W], mm_dt, kind="ExternalInput").ap()
    # wt[ci, t*COUT + co] = weights[co, ci, ky, kx], t = ky*3+kx
    w_in = nc.dram_tensor("wt", [CIN, KH * KW * COUT], mm_dt, kind="ExternalInput").ap()
    out = nc.dram_tensor("out", [BPC, COUT, OH, OW], f32, kind="ExternalOutput").ap()

    with tile.TileContext(nc) as tc:
        with (
            tc.tile_pool(name="wpool", bufs=1) as wpool,
            tc.tile_pool(name="dpool", bufs=2) as dpool,
            tc.tile_pool(name="opool", bufs=2) as opool,
            tc.tile_pool(name="psum", bufs=8, space="PSUM") as ppool,
        ):
            wt = wpool.tile([CIN, KH * KW * COUT], mm_dt)
            nc.sync.dma_start(wt[:], w_in[:])
            for n in range(BPC):
                dt_ = dpool.tile([CIN, H * W], mm_dt)
                nc.sync.dma_start(dt_[:], data_in[n].rearrange("c h w -> c (h w)"))
                d3 = dt_[:].rearrange("c (h w) -> c h w", w=W)
                for g in range(COUT // 128):
                    ot = opool.tile([128, OH * OW], f32)
                    for r, (r0, rows) in enumerate(ROW_GROUPS):
                        ps = ppool.tile([128, rows * OW], f32)
                        for t in range(KH * KW):
                            ky, kx = divmod(t, KW)
                            nc.tensor.matmul(
                                ps[:],
                                wt[:, t * COUT + g * 128 : t * COUT + (g + 1) * 128],
                                d3[:, r0 + ky : r0 + ky + rows, kx : kx + OW],
                                start=(t == 0),
                                stop=(t == KH * KW - 1),
                            )
                        if r % 2 == 0:
                            nc.vector.tensor_copy(
                                ot[:, r0 * OW : (r0 + rows) * OW], ps[:]
                            )
                        else:
                            nc.scalar.copy(ot[:, r0 * OW : (r0 + rows) * OW], ps[:])
                    nc.sync.dma_start(
                        out[n].rearrange("c h w -> c (h w)")[
                            g * 128 : (g + 1) * 128, :
                        ],
                        ot[:],
                    )
    nc.compile()
    return nc


def _get_nc(mm_dtype_name="float32r"):
    if mm_dtype_name not in _cache:
        _cache[mm_dtype_name] = build_nc(mm_dtype_name)
    return _cache[mm_dtype_name]


def kernel(data: np.ndarray, weights: np.ndarray) -> np.ndarray:
    from concourse.bass_utils import run_bass_kernel_spmd

    data = np.ascontiguousarray(np.asarray(data), dtype=np.float32)
    weights = np.asarray(weights, dtype=np.float32)
    # [co, ci, ky, kx] -> [ci, ky, kx, co] -> [ci, 9*co]
    wt = np.ascontiguousarray(weights.transpose(1, 2, 3, 0)).reshape(
        CIN, KH * KW * COUT
    )

    nc = _get_nc()
    in_maps = [
        {"data": data[i * BPC : (i + 1) * BPC], "wt": wt} for i in range(N_CORES)
    ]
    res = run_bass_kernel_spmd(nc, in_maps, core_ids=list(range(N_CORES)))
    return np.concatenate([r["out"] for r in res.results], axis=0)


# revision 2
# speedup vs baseline: 1.0046x; 1.0046x over previous
# Trainium2 Bass kernel for nn_CustomConv2D_57200374448719:
#   data [32,128,64,64] f32 (NCHW) conv weights [256,128,3,3] (OIHW),
#   VALID, stride 1 -> out [32,256,62,62] f32.
#
# Strategy: data-parallel over batch across 8 NeuronCores (4 images per
# core), weights replicated. Per core, implicit GEMM with C_in=128 on the
# SBUF partition axis: for each image / C_out half (128) / group of 8
# output rows, accumulate 9 matmuls (one per 3x3 tap, K=128, N=rows*62)
# into one PSUM bank. The shifted conv windows are strided access
# patterns on the resident [128, 64*64] image tile, so no im2col copy is
# ever materialized. Matmuls run in float32r (bit-identical fp32 in
# memory, reduced-precision multiply at full PE rate; measured error
# identical to the hardware fp32 path); accumulation is fp32 in PSUM.
#
# DMA routing: input images on the sync-engine HWDGE ring, output stores
# on the scalar-engine HWDGE ring (separate FIFO, so stores never
# head-of-line-block loads), replicated weights on the gpsimd SWDGE path
# so they overlap the first image load.
import numpy as np

N_CORES = 8
B, CIN, H, W = 32, 128, 64, 64
COUT, KH, KW = 256, 3, 3
OH, OW = H - KH + 1, W - KW + 1  # 62, 62
BPC = B // N_CORES  # images per core
ROW_GROUPS = [(r0, min(8, OH - r0)) for r0 in range(0, OH, 8)]  # 7x8 + 1x6

_cache = {}


def build_nc(mm_dtype_name="float32r"):
    import concourse.bacc as bacc
    import concourse.mybir as mybir
    import concourse.tile as tile

    mm_dt = getattr(mybir.dt, mm_dtype_name)
    f32 = mybir.dt.float32

    nc = bacc.Bacc("TRN2", target_bir_lowering=False, debug=False, num_devices=N_CORES)
    data_in = nc.dram_tensor("data", [BPC, CIN, H, W], mm_dt, kind="ExternalInput").ap()
    # wt[ci, t*COUT + co] = weights[co, ci, ky, kx], t = ky*3+kx
    w_in = nc.dram_tensor("wt", [CIN, KH * KW * COUT], mm_dt, kind="ExternalInput").ap()
    out = nc.dram_tensor("out", [BPC, COUT, OH, OW], f32, kind="ExternalOutput").ap()

    with tile.TileContext(nc) as tc:
        with (
            tc.tile_pool(name="wpool", bufs=1) as wpool,
            tc.tile_pool(name="dpool", bufs=2) as dpool,
            tc.tile_pool(name="opool", bufs=6) as opool,
            tc.tile_pool(name="psum", bufs=8, space="PSUM") as ppool,
        ):
            wt = wpool.tile([CIN, KH * KW * COUT], mm_dt)
            # SWDGE so the weight load runs concurrently with the first
            # image load on the HWDGE ring.
            nc.gpsimd.dma_start(wt[:], w_in[:])
            for n in range(BPC):
                dt_ = dpool.tile([CIN, H * W], mm_dt)
                nc.sync.dma_start(dt_[:], data_in[n].rearrange("c h w -> c (h w)"))
                d3 = dt_[:].rearrange("c (h w) -> c h w", w=W)
                for g in range(COUT // 128):
                    for r, (r0, rows) in enumerate(ROW_GROUPS):
                        ps = ppool.tile([128, rows * OW], f32)
                        for t in range(KH * KW):
                            ky, kx = divmod(t, KW)
                            nc.tensor.matmul(
                                ps[:],
                                wt[:, t * COUT + g * 128 : t * COUT + (g + 1) * 128],
                                d3[:, r0 + ky : r0 + ky + rows, kx : kx + OW],
                                start=(t == 0),
                                stop=(t == KH * KW - 1),
                            )
                        ot = opool.tile([128, 8 * OW], f32)
                        if r % 2 == 0:
                            nc.vector.tensor_copy(ot[:, : rows * OW], ps[:])
                        else:
                            nc.scalar.copy(ot[:, : rows * OW], ps[:])
                        nc.scalar.dma_start(
                            out[n].rearrange("c h w -> c (h w)")[
                                g * 128 : (g + 1) * 128, r0 * OW : (r0 + rows) * OW
                            ],
                            ot[:, : rows * OW],
                        )
    nc.compile()
    return nc


def _get_nc(mm_dtype_name="float32r"):
    if mm_dtype_name not in _cache:
        _cache[mm_dtype_name] = build_nc(mm_dtype_name)
    return _cache[mm_dtype_name]


def _np_in_dtype(mm_dtype_name):
    if mm_dtype_name == "bfloat16":
        import ml_dtypes

        return ml_dtypes.bfloat16
    return np.float32


def kernel(data: np.ndarray, weights: np.ndarray, _dtype="float32r") -> np.ndarray:
    from concourse.bass_utils import run_bass_kernel_spmd

    np_dt = _np_in_dtype(_dtype)
    data = np.ascontiguousarray(np.asarray(data), dtype=np_dt)
    weights = np.asarray(weights, dtype=np.float32)
    # [co, ci, ky, kx] -> [ci, ky, kx, co] -> [ci, 9*co]
    wt = np.ascontiguousarray(
        weights.transpose(1, 2, 3, 0), dtype=np_dt
    ).reshape(CIN, KH * KW * COUT)

    nc = _get_nc(_dtype)
    in_maps = [
        {"data": data[i * BPC : (i + 1) * BPC], "wt": wt} for i in range(N_CORES)
    ]
    res = run_bass_kernel_spmd(nc, in_maps, core_ids=list(range(N_CORES)))
    return np.concatenate([r["out"] for r in res.results], axis=0)


# revision 3
# speedup vs baseline: 1.0607x; 1.0559x over previous
# Trainium2 Bass kernel for nn_CustomConv2D_57200374448719:
#   data [32,128,64,64] f32 (NCHW) conv weights [256,128,3,3] (OIHW),
#   VALID, stride 1 -> out [32,256,62,62] f32.
#
# Strategy: data-parallel over batch across 8 NeuronCores (4 images per
# core), weights replicated. Per core, implicit GEMM with C_in=128 on the
# SBUF partition axis: for each image / C_out half (128) / group of 8
# output rows, accumulate 9 matmuls (one per 3x3 tap, K=128, N=rows*62)
# into one PSUM bank. The shifted conv windows are strided access
# patterns on the resident image tiles, so no im2col copy is ever
# materialized. Matmuls run in float32r (bit-identical fp32 in memory,
# reduced-precision multiply at full PE rate; measured error identical to
# the hardware fp32 path); accumulation is fp32 in PSUM.
#
# Startup-latency hiding: weights are loaded as two per-co-half chunks
# and each image as two halo'd row-halves, all on the sync-engine HWDGE
# ring, ordered so the first row-group's dependencies (first weight half
# + first image half) land as early as possible. Output stores go on the
# scalar-engine HWDGE ring (separate FIFO) per row-group so they stream
# out during compute.
import numpy as np

N_CORES = 8
B, CIN, H, W = 32, 128, 64, 64
COUT, KH, KW = 256, 3, 3
OH, OW = H - KH + 1, W - KW + 1  # 62, 62
BPC = B // N_CORES  # images per core
ROW_GROUPS = [(r0, min(8, OH - r0)) for r0 in range(0, OH, 8)]  # 7x8 + 1x6
HA = 34  # image half A: rows [0, 34)   (row groups 0-3 need rows 0..33)
HB = H - 32  # image half B: rows [32, 64) (row groups 4-7 need rows 32..63)

_cache = {}


def build_nc(mm_dtype_name="float32r"):
    import concourse.bacc as bacc
    import concourse.mybir as mybir
    import concourse.tile as tile

    mm_dt = getattr(mybir.dt, mm_dtype_name)
    f32 = mybir.dt.float32

    nc = bacc.Bacc("TRN2", target_bir_lowering=False, debug=False, num_devices=N_CORES)
    data_in = nc.dram_tensor("data", [BPC, CIN, H, W], mm_dt, kind="ExternalInput").ap()
    # wt[ci, g*(9*128) + t*128 + co'] = weights[g*128+co', ci, ky, kx], t=ky*3+kx
    w_in = nc.dram_tensor("wt", [CIN, KH * KW * COUT], mm_dt, kind="ExternalInput").ap()
    out = nc.dram_tensor("out", [BPC, COUT, OH, OW], f32, kind="ExternalOutput").ap()
    WG = KH * KW * 128  # columns per co-half weight chunk

    with tile.TileContext(nc) as tc:
        with (
            tc.tile_pool(name="wpool", bufs=1) as wpool,
            tc.tile_pool(name="dpool", bufs=2) as dpool,
            tc.tile_pool(name="opool", bufs=6) as opool,
            tc.tile_pool(name="psum", bufs=8, space="PSUM") as ppool,
        ):
            wts = []
            for g in range(COUT // 128):
                wtg = wpool.tile([CIN, WG], mm_dt, tag=f"wt{g}")
                wts.append(wtg)
            # first weight half first: the very first matmuls need only this
            nc.sync.dma_start(wts[0][:], w_in[:, :WG])
            dtiles = []
            for n in range(BPC):
                da = dpool.tile([CIN, HA * W], mm_dt, tag="da")
                db = dpool.tile([CIN, HB * W], mm_dt, tag="db")
                nc.sync.dma_start(
                    da[:], data_in[n].rearrange("c h w -> c (h w)")[:, : HA * W]
                )
                if n == 0:
                    # second weight half is needed only after ~8 row-groups
                    nc.sync.dma_start(wts[1][:], w_in[:, WG:])
                nc.sync.dma_start(
                    db[:], data_in[n].rearrange("c h w -> c (h w)")[:, 32 * W :]
                )
                dtiles.append((da, db))

            for n in range(BPC):
                da, db = dtiles[n]
                a3 = da[:].rearrange("c (h w) -> c h w", w=W)
                b3 = db[:].rearrange("c (h w) -> c h w", w=W)
                for g in range(COUT // 128):
                    for r, (r0, rows) in enumerate(ROW_GROUPS):
                        h3, hr0 = (a3, r0) if r0 + rows + KH - 1 <= HA else (b3, r0 - 32)
                        ps = ppool.tile([128, rows * OW], f32)
                        for t in range(KH * KW):
                            ky, kx = divmod(t, KW)
                            nc.tensor.matmul(
                                ps[:],
                                wts[g][:, t * 128 : (t + 1) * 128],
                                h3[:, hr0 + ky : hr0 + ky + rows, kx : kx + OW],
                                start=(t == 0),
                                stop=(t == KH * KW - 1),
                            )
                        ot = opool.tile([128, 8 * OW], f32)
                        if r % 2 == 0:
                            nc.vector.tensor_copy(ot[:, : rows * OW], ps[:])
                        else:
                            nc.scalar.copy(ot[:, : rows * OW], ps[:])
                        nc.scalar.dma_start(
                            out[n].rearrange("c h w -> c (h w)")[
                                g * 128 : (g + 1) * 128, r0 * OW : (r0 + rows) * OW
                            ],
                            ot[:, : rows * OW],
                        )
    nc.compile()
    return nc


def _get_nc(mm_dtype_name="float32r"):
    if mm_dtype_name not in _cache:
        _cache[mm_dtype_name] = build_nc(mm_dtype_name)
    return _cache[mm_dtype_name]


def _np_in_dtype(mm_dtype_name):
    if mm_dtype_name == "bfloat16":
        import ml_dtypes

        return ml_dtypes.bfloat16
    return np.float32


def _prep_weights(weights, np_dt):
    # [co, ci, ky, kx] -> [ci][t=ky*3+kx][g][co'] -> [ci][g][t][co'] flat
    w4 = np.asarray(weights, dtype=np.float32).transpose(1, 2, 3, 0)  # ci,ky,kx,co
    w4 = w4.reshape(CIN, KH * KW, COUT // 128, 128).transpose(0, 2, 1, 3)
    return np.ascontiguousarray(w4, dtype=np_dt).reshape(CIN, KH * KW * COUT)


def kernel(data: np.ndarray, weights: np.ndarray, _dtype="float32r") -> np.ndarray:
    from concourse.bass_utils import run_bass_kernel_spmd

    np_dt = _np_in_dtype(_dtype)
    data = np.ascontiguousarray(np.asarray(data), dtype=np_dt)
    wt = _prep_weights(weights, np_dt)

    nc = _get_nc(_dtype)
    in_maps = [
        {"data": data[i * BPC : (i + 1) * BPC], "wt": wt} for i in range(N_CORES)
    ]
    res = run_bass_kernel_spmd(nc, in_maps, core_ids=list(range(N_CORES)))
    return np.concatenate([r["out"] for r in res.results], axis=0)


# revision 5
# speedup vs baseline: 1.0645x; 1.0036x over previous
# Trainium2 Bass kernel for nn_CustomConv2D_57200374448719:
#   data [32,128,64,64] f32 (NCHW) conv weights [256,128,3,3] (OIHW),
#   VALID, stride 1 -> out [32,256,62,62] f32.
#
# Strategy: data-parallel over batch across 8 NeuronCores (4 images per
# core), weights replicated. Per core, implicit GEMM with C_in=128 on the
# SBUF partition axis: for each image / C_out half (128) / group of 8
# output rows, accumulate 9 matmuls (one per 3x3 tap, K=128, N=rows*62)
# into one PSUM bank. The shifted conv windows are strided access
# patterns on the resident image tiles, so no im2col copy is ever
# materialized. Matmuls run in float32r (bit-identical fp32 in memory,
# reduced-precision multiply at full PE rate; measured error identical to
# the hardware fp32 path); accumulation is fp32 in PSUM.
#
# Startup-latency hiding: weights are loaded as two per-co-half chunks
# and each image as two halo'd row-halves, all on the sync-engine HWDGE
# ring, ordered so the first row-group's dependencies (first weight half
# + first image half) land as early as possible. Output stores go on the
# scalar-engine HWDGE ring (separate FIFO) per row-group so they stream
# out during compute.
import numpy as np

N_CORES = 8
B, CIN, H, W = 32, 128, 64, 64
COUT, KH, KW = 256, 3, 3
OH, OW = H - KH + 1, W - KW + 1  # 62, 62
BPC = B // N_CORES  # images per core
ROW_GROUPS = [(r0, min(8, OH - r0)) for r0 in range(0, OH, 8)]  # 7x8 + 1x6
HA = 34  # image half A: rows [0, 34)   (row groups 0-3 need rows 0..33)
HB = H - 32  # image half B: rows [32, 64) (row groups 4-7 need rows 32..63)

_cache = {}


def build_nc(mm_dtype_name="float32r"):
    import concourse.bacc as bacc
    import concourse.mybir as mybir
    import concourse.tile as tile

    mm_dt = getattr(mybir.dt, mm_dtype_name)
    f32 = mybir.dt.float32

    nc = bacc.Bacc("TRN2", target_bir_lowering=False, debug=False, num_devices=N_CORES)
    data_in = nc.dram_tensor("data", [BPC, CIN, H, W], mm_dt, kind="ExternalInput").ap()
    # wt[ci, g*(9*128) + t*128 + co'] = weights[g*128+co', ci, ky, kx], t=ky*3+kx
    w_in = nc.dram_tensor("wt", [CIN, KH * KW * COUT], mm_dt, kind="ExternalInput").ap()
    out = nc.dram_tensor("out", [BPC, COUT, OH, OW], f32, kind="ExternalOutput").ap()
    WG = KH * KW * 128  # columns per co-half weight chunk

    with tile.TileContext(nc) as tc:
        with (
            tc.tile_pool(name="wpool", bufs=1) as wpool,
            tc.tile_pool(name="dpool", bufs=2) as dpool,
            tc.tile_pool(name="opool", bufs=6) as opool,
            tc.tile_pool(name="psum", bufs=8, space="PSUM") as ppool,
        ):
            wts = []
            for g in range(COUT // 128):
                wtg = wpool.tile([CIN, WG], mm_dt, tag=f"wt{g}")
                wts.append(wtg)
            # first weight half first: the very first matmuls need only this
            nc.sync.dma_start(wts[0][:], w_in[:, :WG])
            dtiles = []
            for n in range(BPC):
                # +2 pad columns: the contiguous N=rows*64 matmul windows
                # read up to 2 elements past the last image row (garbage
                # output columns that are never copied out); keep the reads
                # in-bounds and finite.
                da = dpool.tile([CIN, HA * W + 2], mm_dt, tag="da")
                db = dpool.tile([CIN, HB * W + 2], mm_dt, tag="db")
                nc.sync.dma_start(
                    da[:], data_in[n].rearrange("c h w -> c (h w)")[:, : HA * W + 2]
                )
                if n == 0:
                    # second weight half is needed only after ~8 row-groups
                    nc.sync.dma_start(wts[1][:], w_in[:, WG:])
                nc.sync.dma_start(
                    db[:, : HB * W],
                    data_in[n].rearrange("c h w -> c (h w)")[:, 32 * W :],
                )
                # pad columns get arbitrary real data (their outputs are
                # garbage columns that are never copied out)
                nc.sync.dma_start(
                    db[:, HB * W :],
                    data_in[n].rearrange("c h w -> c (h w)")[:, :2],
                )
                dtiles.append((da, db))

            for n in range(BPC):
                da, db = dtiles[n]
                for g in range(COUT // 128):
                    for r, (r0, rows) in enumerate(ROW_GROUPS):
                        ht, hr0 = (da, r0) if r0 + rows + KH - 1 <= HA else (db, r0 - 32)
                        ps = ppool.tile([128, rows * W], f32)
                        for t in range(KH * KW):
                            ky, kx = divmod(t, KW)
                            base = (hr0 + ky) * W + kx
                            nc.tensor.matmul(
                                ps[:],
                                wts[g][:, t * 128 : (t + 1) * 128],
                                ht[:, base : base + rows * W],
                                start=(t == 0),
                                stop=(t == KH * KW - 1),
                            )
                        ot = opool.tile([128, 8 * OW], f32)
                        src = ps[:].rearrange("p (r w) -> p r w", w=W)[:, :, :OW]
                        dst = ot[:, : rows * OW].rearrange(
                            "p (r w) -> p r w", w=OW
                        )
                        if r % 2 == 0:
                            nc.vector.tensor_copy(dst, src)
                        else:
                            nc.scalar.copy(dst, src)
                        nc.scalar.dma_start(
                            out[n].rearrange("c h w -> c (h w)")[
                                g * 128 : (g + 1) * 128, r0 * OW : (r0 + rows) * OW
                            ],
                            ot[:, : rows * OW],
                        )
    nc.compile()
    return nc


def _get_nc(mm_dtype_name="float32r"):
    if mm_dtype_name not in _cache:
        _cache[mm_dtype_name] = build_nc(mm_dtype_name)
    return _cache[mm_dtype_name]


def _np_in_dtype(mm_dtype_name):
    if mm_dtype_name == "bfloat16":
        import ml_dtypes

        return ml_dtypes.bfloat16
    return np.float32


def _prep_weights(weights, np_dt):
    # [co, ci, ky, kx] -> [ci][t=ky*3+kx][g][co'] -> [ci][g][t][co'] flat
    w4 = np.asarray(weights, dtype=np.float32).transpose(1, 2, 3, 0)  # ci,ky,kx,co
    w4 = w4.reshape(CIN, KH * KW, COUT // 128, 128).transpose(0, 2, 1, 3)
    return np.ascontiguousarray(w4, dtype=np_dt).reshape(CIN, KH * KW * COUT)


def kernel(data: np.ndarray, weights: np.ndarray, _dtype="float32r") -> np.ndarray:
    from concourse.bass_utils import run_bass_kernel_spmd

    np_dt = _np_in_dtype(_dtype)
    data = np.ascontiguousarray(np.asarray(data), dtype=np_dt)
    wt = _prep_weights(weights, np_dt)

    nc = _get_nc(_dtype)
    in_maps = [
        {"data": data[i * BPC : (i + 1) * BPC], "wt": wt} for i in range(N_CORES)
    ]
    res = run_bass_kernel_spmd(nc, in_maps, core_ids=list(range(N_CORES)))
    return np.concatenate([r["out"] for r in res.results], axis=0)
